# revision 1
# baseline (speedup 1.0000x reference)
"""BiRNN encoder-decoder Trainium2 kernel.

Strategy: data-parallel over batch (8 cores x 16 rows). All matmuls in
float32r (tf32-like, 1 cyc/row at moving>=256). Recurrences use "layout B":
psum out = (b, H) with state-transpose (hT) as the stationary operand,
weights as the 512-col moving operand; hT is regenerated each step via 4
PE-transposes + one DVE copy. Layer-0 input projections (+all biases) are
precomputed on the host and DMA-prefetched per step; layer-1's input
projection is fused into its recurrence as 8 extra moving-weight matmuls
against the stored layer-0 hT chunks.
"""
import numpy as np
from contextlib import ExitStack

import concourse.bacc as bacc
import concourse.tile as tile
from concourse import mybir
from concourse.bass_utils import run_bass_kernel_spmd

B, T, IN, H, TGT = 128, 128, 3, 512, 32
NC = 8
BC = B // NC  # 16 batch rows per core
F32R = mybir.dt.float32r
F32 = mybir.dt.float32
CH = H // 128  # 4 chunks of the hidden dim

_prog_cache = {}


def _build_program():
    if "nc" in _prog_cache:
        return _prog_cache["nc"]
    nc = bacc.Bacc("TRN2")
    dp = nc.declare_dram_parameter

    xs0_e = dp("xs0", [2, T, BC, H], F32R, isOutput=False)          # per-dir l0 x-proj (+biases), bwd time-reversed
    whh0_e = dp("whh0", [2, CH, 128, H], F32R, isOutput=False)      # enc l0 Whh.T chunks
    whh1_e = dp("whh1", [2, CH, 128, H], F32R, isOutput=False)      # enc l1 Whh.T chunks
    wih1_e = dp("wih1", [2, 2 * CH, 128, H], F32R, isOutput=False)  # enc l1 Wih.T chunks (k<4 fwd-half, k>=4 bwd-half)
    bias1_e = dp("bias1", [2, 1, H], F32R, isOutput=False)          # enc l1 bih+bhh rows
    dwhh_e = dp("dwhh", [4, CH, 128, H], F32R, isOutput=False)      # dec Whh.T chunks
    dwihr_e = dp("dwihr", [3, CH, 128, H], F32R, isOutput=False)    # dec Wihr.T chunks
    dbias_e = dp("dbias", [3, 1, H], F32R, isOutput=False)          # dec l1..3 bias rows
    din0w_e = dp("din0w", [16, H], F32R, isOutput=False)             # [dec_Wih0.T(3,H); bias0(1,H)]
    linw_e = dp("linw", [CH, 128, 16], F32R, isOutput=False)         # lin_W.T chunks
    linb_e = dp("linb", [1, 16], F32R, isOutput=False)
    ident_e = dp("ident", [16, 16], F32R, isOutput=False)
    ones1_e = dp("ones1", [1, 16], F32R, isOutput=False)
    dinit_e = dp("dinit", [BC, 16], F32R, isOutput=False)            # [x_last(16,3), ones]
    out_e = dp("out", [BC, TGT], F32, isOutput=True)

    with tile.TileContext(nc) as tc, ExitStack() as ctx:
        wpool = ctx.enter_context(tc.tile_pool(name="w", bufs=1))
        hts = ctx.enter_context(tc.tile_pool(name="hts", bufs=1))
        sbuf = ctx.enter_context(tc.tile_pool(name="sb", bufs=1))
        pspool = ctx.enter_context(tc.tile_pool(name="ps", bufs=1, space="PSUM"))

        # ---- weights / constants into SBUF ----
        whh0 = wpool.tile([128, 2 * CH * H], F32R)
        whh1 = wpool.tile([128, 2 * CH * H], F32R)
        wih1 = wpool.tile([128, 2 * 2 * CH * H], F32R)
        dwhh = wpool.tile([128, 4 * CH * H], F32R)
        dwihr = wpool.tile([128, 3 * CH * H], F32R)
        for d in range(2):
            for c in range(CH):
                nc.gpsimd.dma_start(whh0[:, (d * CH + c) * H:(d * CH + c + 1) * H], whh0_e[d, c])
                nc.gpsimd.dma_start(whh1[:, (d * CH + c) * H:(d * CH + c + 1) * H], whh1_e[d, c])
            for k in range(2 * CH):
                nc.gpsimd.dma_start(wih1[:, (d * 2 * CH + k) * H:(d * 2 * CH + k + 1) * H], wih1_e[d, k])
        for l in range(4):
            for c in range(CH):
                nc.gpsimd.dma_start(dwhh[:, (l * CH + c) * H:(l * CH + c + 1) * H], dwhh_e[l, c])
        for l in range(3):
            for c in range(CH):
                nc.gpsimd.dma_start(dwihr[:, (l * CH + c) * H:(l * CH + c + 1) * H], dwihr_e[l, c])
        linw = wpool.tile([128, CH * 16], F32R)
        for c in range(CH):
            nc.gpsimd.dma_start(linw[:, 16 * c:16 * (c + 1)], linw_e[c])
        bias1 = wpool.tile([1, 2 * H], F32R)
        dbias = wpool.tile([1, 3 * H], F32R)
        for d in range(2):
            nc.gpsimd.dma_start(bias1[:, d * H:(d + 1) * H], bias1_e[d])
        for l in range(3):
            nc.gpsimd.dma_start(dbias[:, l * H:(l + 1) * H], dbias_e[l])
        din0w = wpool.tile([16, H], F32R)
        nc.gpsimd.dma_start(din0w[:], din0w_e[:])
        linb = wpool.tile([1, 16], F32R)
        nc.gpsimd.dma_start(linb[:], linb_e[:])
        ident = wpool.tile([16, 16], F32R)
        nc.gpsimd.dma_start(ident[:], ident_e[:])
        ones1 = wpool.tile([1, 16], F32R)
        nc.gpsimd.dma_start(ones1[:], ones1_e[:])
        onescol = wpool.tile([BC, 13], F32R)
        nc.gpsimd.dma_start(onescol[:], dinit_e[:, 3:16])

        # per-t l0 hidden-state transposes, both directions (bwd in its own step order)
        hT0 = {d: hts.tile([128, T * 4 * BC], F32R, tag=f"hT0_{d}", name=f"hT0_{d}") for d in range(2)}

        def recur_step(ps_tag, h_tag, whh_tile, whh_off, hT_cur, hT_dst, inject):
            """One RNN step: psum = inject + h @ Whh.T; tanh; rebuild hT."""
            ps = pspool.tile([BC, H], F32, tag=ps_tag, name=ps_tag, bufs=2)
            inject(ps)
            for c in range(CH):
                nc.tensor.matmul(ps[:], hT_cur[:, 16 * c:16 * (c + 1)],
                                 whh_tile[:, whh_off + c * H: whh_off + (c + 1) * H],
                                 start=False, stop=(c == CH - 1))
            h = sbuf.tile([BC, H], F32R, tag="h", name="h", bufs=2)
            nc.scalar.activation(h[:], ps[:], mybir.ActivationFunctionType.Tanh)
            psT = pspool.tile([128, 4 * BC], F32R, tag="psT", name="psT", bufs=2)
            for c in range(CH):
                nc.tensor.transpose(psT[:, 16 * c:16 * (c + 1)], h[:, 128 * c:128 * (c + 1)], ident[:])
            nc.vector.tensor_copy(hT_dst[:], psT[:])
            return h

        # ---- encoder layer 0 (fwd chain d=0, bwd chain d=1, interleaved) ----
        hT_cur = {}
        for t in range(T):
            for d in range(2):
                xs = sbuf.tile([BC, H], F32R, tag="xs", name="xs", bufs=2)
                nc.sync.dma_start(xs[:], xs0_e[d, t])

                def inject(ps, xs=xs):
                    nc.tensor.matmul(ps[:], ident[:], xs[:], start=True, stop=False)
                dst = hT0[d][:, t * 4 * BC:(t + 1) * 4 * BC]
                if t == 0:
                    # h0 = 0: first Whh matmul contributes zeros via zeroed hT? No -
                    # instead: psum = inject only, skip Whh matmuls entirely.
                    ps = pspool.tile([BC, H], F32, tag="psA" if d == 0 else "psB", name="ps0", bufs=2)
                    nc.tensor.matmul(ps[:], ident[:], xs[:], start=True, stop=True)
                    h = sbuf.tile([BC, H], F32R, tag="h", name="h", bufs=2)
                    nc.scalar.activation(h[:], ps[:], mybir.ActivationFunctionType.Tanh)
                    psT = pspool.tile([128, 4 * BC], F32R, tag="psT", name="psT", bufs=2)
                    for c in range(CH):
                        nc.tensor.transpose(psT[:, 16 * c:16 * (c + 1)], h[:, 128 * c:128 * (c + 1)], ident[:])
                    nc.vector.tensor_copy(dst, psT[:])
                else:
                    recur_step("psA" if d == 0 else "psB", f"h0_{d}", whh0, d * CH * H, hT_cur[d], dst, inject)
                hT_cur[d] = dst

        # ---- encoder layer 1 (input = stored l0 hT of both dirs, fused proj) ----
        hT1 = {}
        for t in range(T):
            for d in range(2):
                # kernel-step t of chain d corresponds to real time rt:
                # d=0: rt=t -> fwd l0 slot t, bwd l0 slot T-1-t
                # d=1: rt=T-1-t -> fwd l0 slot T-1-t, bwd l0 slot t
                f_slot = t if d == 0 else T - 1 - t
                b_slot = T - 1 - t if d == 0 else t

                def inject(ps, d=d, f_slot=f_slot, b_slot=b_slot):
                    nc.tensor.matmul(ps[:], ones1[:], bias1[:, d * H:(d + 1) * H], start=True, stop=False)
                    for k in range(2 * CH):
                        src = hT0[0] if k < CH else hT0[1]
                        slot = f_slot if k < CH else b_slot
                        cc = k % CH
                        nc.tensor.matmul(
                            ps[:], src[:, slot * 4 * BC + 16 * cc: slot * 4 * BC + 16 * (cc + 1)],
                            wih1[:, (d * 2 * CH + k) * H:(d * 2 * CH + k + 1) * H],
                            start=False, stop=False)
                dst = hts.tile([128, 4 * BC], F32R, tag=f"hT1_{d}", name=f"hT1_{d}", bufs=2)
                if t == 0:
                    # no Whh term at t=0 (h0 = 0): close the group on the last inject mm
                    ps = pspool.tile([BC, H], F32, tag="psA" if d == 0 else "psB", name="ps1", bufs=2)
                    nc.tensor.matmul(ps[:], ones1[:], bias1[:, d * H:(d + 1) * H], start=True, stop=False)
                    for k in range(2 * CH):
                        src = hT0[0] if k < CH else hT0[1]
                        slot = f_slot if k < CH else b_slot
                        cc = k % CH
                        nc.tensor.matmul(
                            ps[:], src[:, slot * 4 * BC + 16 * cc: slot * 4 * BC + 16 * (cc + 1)],
                            wih1[:, (d * 2 * CH + k) * H:(d * 2 * CH + k + 1) * H],
                            start=False, stop=(k == 2 * CH - 1))
                    h = sbuf.tile([BC, H], F32R, tag="h", name="h", bufs=2)
                    nc.scalar.activation(h[:], ps[:], mybir.ActivationFunctionType.Tanh)
                    psT = pspool.tile([128, 4 * BC], F32R, tag="psT", name="psT", bufs=2)
                    for c in range(CH):
                        nc.tensor.transpose(psT[:, 16 * c:16 * (c + 1)], h[:, 128 * c:128 * (c + 1)], ident[:])
                    nc.vector.tensor_copy(dst[:], psT[:])
                else:
                    recur_step("psA" if d == 0 else "psB", f"h1_{d}", whh1, d * CH * H, hT1[d], dst[:], inject)
                hT1[d] = dst[:]

        # ---- decoder: 4-layer stack, 32 autoregressive steps ----
        hT_dec = {0: hT0[0][:, (T - 1) * 4 * BC: T * 4 * BC],  # hf0
                  1: hT0[1][:, (T - 1) * 4 * BC: T * 4 * BC],  # hb0 (its last kernel step = real t0)
                  2: hT1[0], 3: hT1[1]}
        xin = sbuf.tile([BC, 16], F32R, tag="xin", name="xin", bufs=2)
        nc.sync.dma_start(xin[:], dinit_e[:])
        outcol = sbuf.tile([BC, TGT], F32, tag="outcol", name="outcol")

        for t in range(TGT):
            # transpose current input cols (16,4) -> (4,16) for the l0 inject
            psx = pspool.tile([16, 16], F32R, tag="psS", name="psxT", bufs=2)
            nc.tensor.transpose(psx[:], xin[:], ident[:])
            xinT = sbuf.tile([16, 16], F32R, tag="xinT", name="xinT")
            nc.vector.tensor_copy(xinT[:], psx[:])

            h_below = None
            for l in range(4):
                ps = pspool.tile([BC, H], F32, tag="psA", name="ps_dec", bufs=2)
                if l == 0:
                    nc.tensor.matmul(ps[:], xinT[:], din0w[:], start=True, stop=False)
                else:
                    nc.tensor.matmul(ps[:], ones1[:], dbias[:, (l - 1) * H:l * H], start=True, stop=False)
                    for c in range(CH):
                        nc.tensor.matmul(ps[:], h_below[:, 16 * c:16 * (c + 1)],
                                         dwihr[:, ((l - 1) * CH + c) * H:((l - 1) * CH + c + 1) * H],
                                         start=False, stop=False)
                for c in range(CH):
                    nc.tensor.matmul(ps[:], hT_dec[l][:, 16 * c:16 * (c + 1)],
                                     dwhh[:, (l * CH + c) * H:(l * CH + c + 1) * H],
                                     start=False, stop=(c == CH - 1))
                h = sbuf.tile([BC, H], F32R, tag="h", name="h", bufs=2)
                nc.scalar.activation(h[:], ps[:], mybir.ActivationFunctionType.Tanh)
                psT = pspool.tile([128, 4 * BC], F32R, tag="psT", name="psT", bufs=2)
                for c in range(CH):
                    nc.tensor.transpose(psT[:, 16 * c:16 * (c + 1)], h[:, 128 * c:128 * (c + 1)], ident[:])
                hT_new = hts.tile([128, 4 * BC], F32R, tag=f"hTd_{l}", name=f"hTd_{l}", bufs=2)
                nc.vector.tensor_copy(hT_new[:], psT[:])
                hT_dec[l] = hT_new[:]
                h_below = hT_new[:]

            # linear head: out = h3 @ lin_W.T + lin_b  -> (16,1)
            pso = pspool.tile([BC, 16], F32, tag="psS", name="ps_o", bufs=2)
            nc.tensor.matmul(pso[:], ones1[:], linb[:], start=True, stop=False)
            for c in range(CH):
                nc.tensor.matmul(pso[:], hT_dec[3][:, 16 * c:16 * (c + 1)], linw[:, 16 * c:16 * (c + 1)],
                                 start=False, stop=(c == CH - 1))
            # next input columns: [o0, x0-o0, x1-(x0-o0), 1]
            xin_new = sbuf.tile([BC, 16], F32R, tag="xin", name="xin", bufs=2)
            nc.vector.tensor_copy(xin_new[:, 0:1], pso[:, 0:1])
            nc.vector.tensor_tensor(xin_new[:, 1:2], xin[:, 0:1], xin_new[:, 0:1], mybir.AluOpType.subtract)
            nc.vector.tensor_tensor(xin_new[:, 2:3], xin[:, 1:2], xin_new[:, 1:2], mybir.AluOpType.subtract)
            nc.vector.tensor_copy(xin_new[:, 3:16], onescol[:])
            nc.vector.tensor_copy(outcol[:, t:t + 1], pso[:, 0:1])
            xin = xin_new

        nc.sync.dma_start(out_e[:], outcol[:])

    nc.compile()
    _prog_cache["nc"] = nc
    return nc


def kernel(x, y, enc_Wih0, enc_Whh0, enc_Wih1, enc_Whh1, enc_bih, enc_bhh,
           dec_Wih0, dec_Wihr, dec_Whh, dec_bih, dec_bhh, lin_W, lin_b,
           target_len, teacher_forcing_ratio):
    x = np.asarray(x, np.float32)
    f = np.float32

    def chunksT(W):  # (H,K) -> (K//128, 128, H) chunks of W.T
        WT = np.ascontiguousarray(W.T.astype(f))
        return WT.reshape(WT.shape[0] // 128, 128, WT.shape[1])

    whh0 = np.stack([chunksT(np.asarray(enc_Whh0)[d]) for d in range(2)])
    whh1 = np.stack([chunksT(np.asarray(enc_Whh1)[d]) for d in range(2)])
    wih1 = np.stack([chunksT(np.asarray(enc_Wih1)[d]) for d in range(2)])
    dwhh = np.stack([chunksT(np.asarray(dec_Whh)[l]) for l in range(4)])
    dwihr = np.stack([chunksT(np.asarray(dec_Wihr)[l]) for l in range(3)])
    bias1 = np.stack([(np.asarray(enc_bih)[1, d] + np.asarray(enc_bhh)[1, d]).astype(f)[None, :]
                      for d in range(2)])
    dbias = np.stack([(np.asarray(dec_bih)[l] + np.asarray(dec_bhh)[l]).astype(f)[None, :]
                      for l in range(1, 4)])
    din0w = np.zeros((16, H), f)
    din0w[:3] = np.asarray(dec_Wih0, f).T
    din0w[3] = (np.asarray(dec_bih)[0] + np.asarray(dec_bhh)[0]).astype(f)
    linw = np.zeros((CH, 128, 16), f)
    linw[:, :, 0] = np.asarray(lin_W, f).T.reshape(CH, 128)
    linb = np.zeros((1, 16), f)
    linb[0, 0] = np.asarray(lin_b, f).reshape(())
    ident = np.eye(16, dtype=f)
    ones1 = np.ones((1, 16), f)

    nc = _build_program()
    in_maps = []
    for c in range(NC):
        xc = x[c * BC:(c + 1) * BC]  # (16, T, 3)
        xs0 = np.empty((2, T, BC, H), f)
        for d in range(2):
            W = np.asarray(enc_Wih0)[d].astype(f)
            b = (np.asarray(enc_bih)[0, d] + np.asarray(enc_bhh)[0, d]).astype(f)
            proj = np.einsum('bti,hi->tbh', xc, W) + b  # (T, 16, H)
            xs0[d] = proj if d == 0 else proj[::-1]
        dinit = np.zeros((BC, 16), f)
        dinit[:, :3] = xc[:, -1, :]
        dinit[:, 3] = 1.0
        in_maps.append({
            "xs0": xs0, "whh0": whh0, "whh1": whh1, "wih1": wih1, "bias1": bias1,
            "dwhh": dwhh, "dwihr": dwihr, "dbias": dbias, "din0w": din0w,
            "linw": linw, "linb": linb, "ident": ident, "ones1": ones1, "dinit": dinit,
        })
    res = run_bass_kernel_spmd(nc, in_maps, list(range(NC)))
    out = np.concatenate([res.results[c]["out"] for c in range(NC)], 0)
    return out.reshape(B, TGT, 1).astype(np.float32)



# revision 7
# speedup vs baseline: 4.5583x; 4.5583x over previous
"""BiRNN encoder-decoder Trainium2 kernel, feature-major layout.

Data-parallel over batch (8 cores x 16 rows). All state is kept
feature-major: h lives in SBUF as [128 (H-chunk), 16 (batch)] fp16 columns,
weights are the PE stationary operand ([k-chunk, n-chunk] tiles of W.T) and
the state is the moving operand, so each recurrent matmul's cost scales with
the 16-wide batch (free size) instead of the 512-wide hidden dim. No
transposes anywhere: the PSUM output [128n, 16b] of one step is exactly the
moving layout the next step needs; tanh evacuates PSUM->SBUF directly.

Decoder feedback is algebraically folded into the layer-0 matmul: with
o0 = lin.h3 + lb and nxt = [o0, x0-o0, x1-x0+o0], layer-0's next-step input
projection W0.nxt becomes A.h3 + B2.[x0;x1] + c0 with A = W0.N.lin (rank-1,
precomputed on host), so the head+feedback hop disappears from the serial
chain; the visible outputs are recovered after the loop by one batched GEMM
over the stored h3 states.
"""
import numpy as np
from contextlib import ExitStack

import concourse.bacc as bacc
import concourse.tile as tile
from concourse import mybir
from concourse.bass_utils import run_bass_kernel_spmd

B, T, IN, H, TGT = 128, 128, 3, 512, 32
NC = 8
BC = B // NC          # 16 batch rows per core
CH = H // 128         # 4 chunks of the hidden dim
F16 = mybir.dt.float16
F32 = mybir.dt.float32
Tanh = mybir.ActivationFunctionType.Tanh

# smalls tile column offsets (fp16 [128, C_SMALL])
B1D0, B1D1 = 0, 512            # enc l1 bias rows (row 0)
DB = 1024                      # dec l1..3 bias rows (row 0), 512 each
C0 = 2560                      # dec l0 const row (row 0)
CS = 3072                      # xin const row [1,2] (row 0)
B2C = 3074                     # dec l0 xin coeffs [2,512] (rows 0-1)
S2C = 3586                     # xin xin-coeffs [2,2] (rows 0-1)
DIN0 = 3588                    # dec l0 t=0 stationary [4,512] (rows 0-3)
XQ = 4100                      # per-core x-init [4,16] rows (x0,x1,1,x2)
LINC = 4116                    # lin head chunks [128,4]
WX2 = 4120                     # xin h3-coeff chunks [128,8]
ONES = 4128                    # all-ones [128,16]
IDC = 4144                     # identity [128,128]
LB = 4272                      # lin_b scalar (row 0)
C_SMALL = 4274

_prog_cache = {}


def _build_program():
    if "nc" in _prog_cache:
        return _prog_cache["nc"]
    nc = bacc.Bacc("TRN2")
    dp = nc.declare_dram_parameter

    whh0_e = dp("whh0", [128, 2 * 2048], F16, isOutput=False)
    xs0_e = dp("xs0", [2, 128, T * 4 * BC], F16, isOutput=False)
    wenc1_e = dp("wenc1", [128, 2 * 2048 + 2 * 4096], F16, isOutput=False)
    wdec_e = dp("wdec", [128, 8 * 2048], F16, isOutput=False)
    smalls_e = dp("smalls", [128, C_SMALL], F16, isOutput=False)
    out_e = dp("out", [1, TGT * BC], F32, isOutput=True)

    SW = T * 4 * BC  # 8192 cols per direction

    with tile.TileContext(nc) as tc, ExitStack() as ctx:
        wpool = ctx.enter_context(tc.tile_pool(name="w", bufs=1))
        hpool = ctx.enter_context(tc.tile_pool(name="h", bufs=1))
        pspool = ctx.enter_context(tc.tile_pool(name="ps", bufs=1, space="PSUM"))

        whh0s = wpool.tile([128, 2 * 2048], F16)   # enc l0 Whh.T (d,kc,nc)
        xs0 = wpool.tile([128, 2 * SW], F16)       # l0 x-proj(+bias), feature-major
        wenc1 = wpool.tile([128, 2 * 2048 + 2 * 4096], F16)  # whh1 (2) | wih1 (2)
        wdec = wpool.tile([128, 8 * 2048], F16)    # dwhh(4) | dwihr(3) | A
        smalls = wpool.tile([128, C_SMALL], F16)
        hbuf0 = {d: wpool.tile([128, SW], F16, name=f"hbuf0_{d}") for d in range(2)}
        hbuf3 = wpool.tile([128, TGT * 4 * BC], F16)   # dec l3 states per t

        # critical-path DMAs first (sync queue), bulk weights behind (gpsimd)
        nc.sync.dma_start(whh0s[:], whh0_e[:])
        NXC = 8  # xs chunks per direction
        xcw = SW // NXC
        for i in range(NXC):
            for d in range(2):
                nc.sync.dma_start(xs0[:, d * SW + i * xcw:d * SW + (i + 1) * xcw],
                                  xs0_e[d, :, i * xcw:(i + 1) * xcw])
        nc.gpsimd.dma_start(smalls[:], smalls_e[:])
        nc.gpsimd.dma_start(wenc1[:], wenc1_e[:])
        nc.gpsimd.dma_start(wdec[:], wdec_e[:])

        ident = smalls[:, IDC:IDC + 128]
        ones1 = smalls[0:1, ONES:ONES + 16]

        def mm(ps_ap, lhsT_ap, rhs_ap, start, stop):
            nc.tensor.matmul(ps_ap, lhsT_ap, rhs_ap, start=start, stop=stop)

        # ---- encoder layer 0: fwd (d=0) and bwd (d=1) chains interleaved ----
        # h state for (d, step t) lives at hbuf0[d][:, t*64:(t+1)*64]
        for t in range(T):
            for d in range(2):
                ps = pspool.tile([128, 512], F32, tag=f"psE{d}", name=f"psE{d}", bufs=2)
                xsl = xs0[:, d * SW + t * 64:d * SW + (t + 1) * 64]
                mm(ps[:, 0:64], ident, xsl, True, t == 0)
                if t > 0:
                    hprev = hbuf0[d][:, (t - 1) * 64:t * 64]
                    for kc in range(CH):
                        for nb in range(CH):
                            mm(ps[:, 16 * nb:16 * (nb + 1)],
                               whh0s[:, d * 2048 + kc * 512 + nb * 128:d * 2048 + kc * 512 + (nb + 1) * 128],
                               hprev[:, 16 * kc:16 * (kc + 1)],
                               False, kc == CH - 1 and nb == CH - 1)
                nc.scalar.activation(hbuf0[d][:, t * 64:(t + 1) * 64], ps[:, 0:64], Tanh)

        # ---- encoder layer 1: fused input projection from hbuf0 ----
        WIH1 = 2 * 2048  # offset of wih1 region inside wenc1
        e1h = {}
        for t in range(T):
            for d in range(2):
                f_slot = t if d == 0 else T - 1 - t
                b_slot = T - 1 - t if d == 0 else t
                ps = pspool.tile([128, 512], F32, tag=f"psE{d}", name=f"psF{d}", bufs=2)
                for nb in range(CH):  # bias rows
                    mm(ps[:, 16 * nb:16 * (nb + 1)],
                       smalls[0:1, d * 512 + nb * 128:d * 512 + (nb + 1) * 128],
                       ones1, nb == 0, False)
                for k8 in range(2 * CH):  # input projection (2H contraction)
                    src = hbuf0[0] if k8 < CH else hbuf0[1]
                    slot = f_slot if k8 < CH else b_slot
                    rhs = src[:, slot * 64 + 16 * (k8 % CH):slot * 64 + 16 * (k8 % CH + 1)]
                    for nb in range(CH):
                        mm(ps[:, 16 * nb:16 * (nb + 1)],
                           wenc1[:, WIH1 + d * 4096 + k8 * 512 + nb * 128:WIH1 + d * 4096 + k8 * 512 + (nb + 1) * 128],
                           rhs, False,
                           t == 0 and k8 == 2 * CH - 1 and nb == CH - 1)
                if t > 0:
                    hprev = e1h[d][:, 0:64]
                    for kc in range(CH):
                        for nb in range(CH):
                            mm(ps[:, 16 * nb:16 * (nb + 1)],
                               wenc1[:, d * 2048 + kc * 512 + nb * 128:d * 2048 + kc * 512 + (nb + 1) * 128],
                               hprev[:, 16 * kc:16 * (kc + 1)],
                               False, kc == CH - 1 and nb == CH - 1)
                hnew = hpool.tile([128, 64], F16, tag=f"e1_{d}", name=f"e1_{d}", bufs=2)
                nc.scalar.activation(hnew[:], ps[:, 0:64], Tanh)
                e1h[d] = hnew

        # ---- decoder: 4-layer stack, 32 autoregressive steps ----
        DWIHR = 4 * 2048
        AOFF = 7 * 2048
        hdec = {0: hbuf0[0][:, (T - 1) * 64:T * 64],
                1: hbuf0[1][:, (T - 1) * 64:T * 64],
                2: e1h[0][:, 0:64], 3: e1h[1][:, 0:64]}
        xq = smalls[0:3, XQ:XQ + 16]  # rows (x0, x1, 1)
        for t in range(TGT):
            # layer 0
            ps = pspool.tile([128, 512], F32, tag="psD", name="psD", bufs=2)
            if t == 0:
                for nb in range(CH):
                    mm(ps[:, 16 * nb:16 * (nb + 1)],
                       smalls[0:4, DIN0 + nb * 128:DIN0 + (nb + 1) * 128],
                       smalls[0:4, XQ:XQ + 16], nb == 0, False)
            else:
                h3p = hbuf3[:, (t - 1) * 64:t * 64]
                for kc in range(CH):  # A @ h3
                    for nb in range(CH):
                        mm(ps[:, 16 * nb:16 * (nb + 1)],
                           wdec[:, AOFF + kc * 512 + nb * 128:AOFF + kc * 512 + (nb + 1) * 128],
                           h3p[:, 16 * kc:16 * (kc + 1)],
                           kc == 0 and nb == 0, False)
                for nb in range(CH):  # B2 @ [x0;x1] + c0
                    mm(ps[:, 16 * nb:16 * (nb + 1)],
                       smalls[0:2, B2C + nb * 128:B2C + (nb + 1) * 128],
                       xq[0:2, :], False, False)
                    mm(ps[:, 16 * nb:16 * (nb + 1)],
                       smalls[0:1, C0 + nb * 128:C0 + (nb + 1) * 128],
                       ones1, False, False)
            for kc in range(CH):  # Whh0 @ h0_prev
                for nb in range(CH):
                    mm(ps[:, 16 * nb:16 * (nb + 1)],
                       wdec[:, kc * 512 + nb * 128:kc * 512 + (nb + 1) * 128],
                       hdec[0][:, 16 * kc:16 * (kc + 1)],
                       False, kc == CH - 1 and nb == CH - 1)
            h0 = hpool.tile([128, 64], F16, tag="hd0", name="hd0", bufs=2)
            nc.scalar.activation(h0[:], ps[:, 0:64], Tanh)
            hdec[0] = h0[:]

            # xin update for next step: [x0;x1]_{t+1} from h3_t (issued later,
            # after h3_t exists) -- see below
            # layers 1..3
            for l in range(1, 4):
                ps = pspool.tile([128, 512], F32, tag="psD", name="psD", bufs=2)
                for nb in range(CH):  # bias
                    mm(ps[:, 16 * nb:16 * (nb + 1)],
                       smalls[0:1, DB + (l - 1) * 512 + nb * 128:DB + (l - 1) * 512 + (nb + 1) * 128],
                       ones1, nb == 0, False)
                for kc in range(CH):  # Wih @ h_below
                    for nb in range(CH):
                        mm(ps[:, 16 * nb:16 * (nb + 1)],
                           wdec[:, DWIHR + (l - 1) * 2048 + kc * 512 + nb * 128:DWIHR + (l - 1) * 2048 + kc * 512 + (nb + 1) * 128],
                           hdec[l - 1][:, 16 * kc:16 * (kc + 1)], False, False)
                for kc in range(CH):  # Whh @ h_l_prev
                    for nb in range(CH):
                        mm(ps[:, 16 * nb:16 * (nb + 1)],
                           wdec[:, l * 2048 + kc * 512 + nb * 128:l * 2048 + kc * 512 + (nb + 1) * 128],
                           hdec[l][:, 16 * kc:16 * (kc + 1)],
                           False, kc == CH - 1 and nb == CH - 1)
                if l == 3:
                    nc.scalar.activation(hbuf3[:, t * 64:(t + 1) * 64], ps[:, 0:64], Tanh)
                    hdec[3] = hbuf3[:, t * 64:(t + 1) * 64]
                else:
                    hl = hpool.tile([128, 64], F16, tag=f"hd{l}", name=f"hd{l}", bufs=2)
                    nc.scalar.activation(hl[:], ps[:, 0:64], Tanh)
                    hdec[l] = hl[:]

            if 1 <= t < TGT - 1:
                # xin01_t = Wx2.h3_{t-1} + S2.xin01_{t-1} + cS; h3_{t-1} has
                # been ready since last step, so this chain is off the
                # critical path with a full step of slack.
                px = pspool.tile([128, 512], F32, tag="psX", name="psX", bufs=1)
                for kc in range(CH):
                    mm(px[0:2, 0:16],
                       smalls[:, WX2 + 2 * kc:WX2 + 2 * (kc + 1)],
                       hbuf3[:, (t - 1) * 64 + 16 * kc:(t - 1) * 64 + 16 * (kc + 1)],
                       kc == 0, False)
                mm(px[0:2, 0:16], smalls[0:2, S2C:S2C + 2], xq[0:2, :], False, False)
                mm(px[0:2, 0:16], smalls[0:1, CS:CS + 2], ones1, False, True)
                xnew = hpool.tile([2, 16], F16, tag="xin", name="xin", bufs=2)
                nc.vector.tensor_copy(xnew[:], px[0:2, 0:16])
                xq = xnew[:]

        # ---- head: o0_t = lin.h3_t + lb, all t in one batched group ----
        ph = pspool.tile([128, 512], F32, tag="psX", name="psH", bufs=1)
        for t in range(TGT):
            for kc in range(CH):
                mm(ph[0:1, 16 * t:16 * (t + 1)],
                   smalls[:, LINC + kc:LINC + kc + 1],
                   hbuf3[:, t * 64 + 16 * kc:t * 64 + 16 * (kc + 1)],
                   t == 0 and kc == 0, t == TGT - 1 and kc == CH - 1)
        outt = hpool.tile([1, TGT * BC], F32, tag="out", name="out")
        nc.scalar.activation(outt[:], ph[0:1, 0:TGT * BC],
                             mybir.ActivationFunctionType.Identity,
                             bias=smalls[0:1, LB:LB + 1])
        nc.sync.dma_start(out_e[:], outt[:])

    nc.compile()
    _prog_cache["nc"] = nc
    return nc


def _statT(W):
    """W (N,K), h_new = W @ h -> stationary tile [128, (K//128)*N]:
    chunk kc at cols [kc*N:(kc+1)*N] holds W.T[128*kc:128*(kc+1), :]."""
    W = np.asarray(W, np.float32)
    N, K = W.shape
    WT = np.ascontiguousarray(W.T)
    return WT.reshape(K // 128, 128, N).transpose(1, 0, 2).reshape(128, (K // 128) * N)


def kernel(x, y, enc_Wih0, enc_Whh0, enc_Wih1, enc_Whh1, enc_bih, enc_bhh,
           dec_Wih0, dec_Wihr, dec_Whh, dec_bih, dec_bhh, lin_W, lin_b,
           target_len, teacher_forcing_ratio):
    f, h16 = np.float32, np.float16
    x = np.asarray(x, f)
    enc_Wih0, enc_Whh0 = np.asarray(enc_Wih0, f), np.asarray(enc_Whh0, f)
    enc_Wih1, enc_Whh1 = np.asarray(enc_Wih1, f), np.asarray(enc_Whh1, f)
    enc_bih, enc_bhh = np.asarray(enc_bih, f), np.asarray(enc_bhh, f)
    dec_Wih0, dec_Wihr = np.asarray(dec_Wih0, f), np.asarray(dec_Wihr, f)
    dec_Whh = np.asarray(dec_Whh, f)
    dec_bih, dec_bhh = np.asarray(dec_bih, f), np.asarray(dec_bhh, f)
    lin_W = np.asarray(lin_W, f)
    lb = float(np.asarray(lin_b, f).reshape(()))

    whh0 = np.concatenate([_statT(enc_Whh0[d]) for d in range(2)], 1).astype(h16)
    wenc1 = np.concatenate([_statT(enc_Whh1[d]) for d in range(2)]
                           + [_statT(enc_Wih1[d]) for d in range(2)], 1).astype(h16)

    W0, linv = dec_Wih0, lin_W[0]  # (512,3), (512,)
    Nv = np.array([1.0, -1.0, 1.0], f)
    A = np.outer(W0 @ Nv, linv)                      # (512,512)
    b0tot = dec_bih[0] + dec_bhh[0]
    c0 = (W0 @ Nv) * lb + b0tot                      # (512,)
    B2 = np.stack([W0[:, 1] - W0[:, 2], W0[:, 2]])   # (2,512): x0,x1 coeffs
    wdec = np.concatenate([_statT(dec_Whh[l]) for l in range(4)]
                          + [_statT(dec_Wihr[l]) for l in range(3)]
                          + [_statT(A)], 1).astype(h16)

    smalls = np.zeros((128, C_SMALL), f)
    for d in range(2):
        smalls[0, d * 512:(d + 1) * 512] = enc_bih[1, d] + enc_bhh[1, d]
    for l in range(1, 4):
        smalls[0, DB + (l - 1) * 512:DB + l * 512] = dec_bih[l] + dec_bhh[l]
    smalls[0, C0:C0 + 512] = c0
    smalls[0, CS:CS + 2] = [lb, -lb]
    smalls[0:2, B2C:B2C + 512] = B2
    smalls[0:2, S2C:S2C + 2] = np.array([[0, 1], [0, 0]], f)
    din0q = np.zeros((4, 512), f)   # rows match xq rows (x0, x1, 1, x2)
    din0q[0], din0q[1], din0q[3] = W0[:, 0], W0[:, 1], W0[:, 2]
    din0q[2] = b0tot
    smalls[0:4, DIN0:DIN0 + 512] = din0q
    smalls[:, LINC:LINC + 4] = linv.reshape(4, 128).T
    wx2 = np.stack([linv, -linv])                    # (2,512)
    smalls[:, WX2:WX2 + 8] = wx2.T.reshape(4, 128, 2).transpose(1, 0, 2).reshape(128, 8)
    smalls[:, ONES:ONES + 16] = 1.0
    smalls[:, IDC:IDC + 128] = np.eye(128, dtype=f)
    smalls[0, LB] = lb

    nc = _build_program()

    in_maps = []
    for c in range(NC):
        xc = x[c * BC:(c + 1) * BC]  # (16, T, 3)
        xs0 = np.empty((2, 128, T * 4 * BC), h16)
        for d in range(2):
            W = enc_Wih0[d]
            b = enc_bih[0, d] + enc_bhh[0, d]
            proj = np.einsum('bti,hi->tbh', xc, W) + b   # (T, 16, 512)
            if d == 1:
                proj = proj[::-1]
            xs0[d] = proj.reshape(T, BC, 4, 128).transpose(3, 0, 2, 1).reshape(128, T * 4 * BC)
        sm = smalls.copy()
        sm[0:2, XQ:XQ + 16] = xc[:, -1, 0:2].T
        sm[2, XQ:XQ + 16] = 1.0
        sm[3, XQ:XQ + 16] = xc[:, -1, 2]
        in_maps.append({
            "whh0": whh0, "xs0": xs0, "wenc1": wenc1, "wdec": wdec,
            "smalls": sm.astype(h16),
        })
    res = run_bass_kernel_spmd(nc, in_maps, list(range(NC)))
    out = np.stack([res.results[c]["out"].reshape(TGT, BC).T for c in range(NC)])
    return out.reshape(B, TGT, 1).astype(np.float32)
